# revision 35
# baseline (speedup 1.0000x reference)
"""Trainium2 Bass kernel for AdvancedKANLayer.

Math (per reference):
  xn = tanh(x)                                  (B, IN)
  d_g = |xn - g|                                for 8 grid points g
  f(d) = 2*(1-d)+^3 - 8*(0.5-d)+^3              (piecewise-cubic B-spline basis)
  out[b,o] = sum_{i,g} f(d_g[b,i]) * sw[o,i,g] + 0.1 * xn @ ba.T

Device formulation (per core, batch-sharded 8 ways, b_loc=512), with
a = (1-|u|)+ (tent, u = xn-g) and b = (a-0.5)+ so that f = 2a^3 - 8b^3:
  - edge grid points (g=-1, +1): tent collapses, a = relu(-+xn),
    b = relu(-+xn-0.5) -- one DVE tensor_scalar (4x mode) each.
  - interior g < k_abs (ACT Abs route): d = Abs(xn-g) on ScalarE, then
    -a = min(d-1, 0), -b = min(d-0.5, 0) via DVE tensor_scalar (4x).
  - interior g >= k_abs (min-of-relus route): r1 = relu(-xn+g+1),
    r2 = relu(xn-g+1) (DVE ts 4x), a = min(r1, r2) (tt 2x),
    b = relu(a-0.5) (ts 4x).
  sA = Square(sqrt(2)*(+-a)) = 2a^2, sB = Square(sqrt(8)*(+-b)) = 8b^2 on
  ScalarE (all constant folds ride the free Square input scale), then
  CU = SQ*M (tt 2x) gives +-2a^3 / +-8b^3 and F = CU_A - CU_B = +-f (tt 2x).
  Per-channel signs are absorbed into the weights.
  out = W2.T @ [F channels (8 per i-tile), xn channel] -- single fp16 PE
  contraction, K = 4*(8+1)*128 = 4608, 144 matmuls of N=512 accumulating
  in 4 PSUM banks. W2 = [+-sw | 0.1*ba].

The work per i-tile is split into two 4-grid halves so PE bursts interleave
with the elementwise pipeline (keeps the PE HAM warm-ish). Weight k-tiles
are packed 4-per-DMA in consumption order so the first matmuls unblock
early. Layout: i on partitions (4 tiles of 128), b on free dim (512). x is
passed transposed per core: xT[i, b]. Output is [o, b] per core, gathered +
transposed on host. Elementwise tensors fp16; matmul fp16 -> fp32 PSUM.
Measured: ~70us/core HW exec, rel err ~7e-4 vs the fp32 reference.
"""

import sys

if "/opt/trn_rl_repo" not in sys.path:
    sys.path.insert(0, "/opt/trn_rl_repo")

import numpy as np

IN_F = 512
OUT_F = 512
GRID = 8
BATCH = 4096
NCORES = 8
B_LOC = BATCH // NCORES  # 512
NT = IN_F // 128         # 4 i-tiles
NO = OUT_F // 128        # 4 o-tiles
NCH = GRID + 1           # 8 basis channels + 1 xn channel per i-tile
NK = NT * NCH            # 36 k-tiles of 128

CFG = {
    "k_abs": 5,          # g's 0..k_abs-1 use ACT Abs route; rest use DVE min-of-relus
    "sq_on_act": 2,      # 0..2 of the two Square layers on ScalarE (rest DVE stt)
    "copy_on_act": True,  # PSUM->SBUF output copies on ScalarE
}

# Weight k-tiles are DMA'd packed PACK-at-a-time (PACK*1KB contiguous DRAM rows)
# in the order the matmuls consume them: per i-tile, the xn channel first.
PACK = 4
CONSUME_ORDER = []
for _t in range(NT):
    CONSUME_ORDER.append(_t * NCH + GRID)
    CONSUME_ORDER.extend(_t * NCH + _g for _g in range(GRID))
KT_SLOT = {kt: (j // PACK, j % PACK) for j, kt in enumerate(CONSUME_ORDER)}

_CACHE = {}

SQRT2 = float(np.sqrt(2.0))
SQRT8 = float(np.sqrt(8.0))


def _routes(grid_vals, k_abs):
    """Per-grid-point compute route and channel sign (shared by the kernel
    builder and the weight prep so they can never disagree).

    The edge shortcut relies on grid[0] = -1 and grid[-1] = +1 (with
    xn = tanh(x) strictly inside (-1, 1)); fall back to the Abs route if a
    different grid ever shows up.
    """
    route = {}
    sign = {}
    for g in range(GRID):
        if g == 0 and abs(float(grid_vals[0]) + 1.0) < 1e-6:
            route[g], sign[g] = "e0", -1.0
        elif g == GRID - 1 and abs(float(grid_vals[GRID - 1]) - 1.0) < 1e-6:
            route[g], sign[g] = "e7", 1.0
        elif 0 < g < min(k_abs, GRID - 1):
            route[g], sign[g] = "abs", -1.0
        else:
            route[g], sign[g] = "min", 1.0
    return route, sign


def _build(grid_vals, cfg):
    import concourse.tile as tile
    import concourse.mybir as mybir
    from concourse import bacc

    dt = mybir.dt
    f16 = dt.float16
    f32 = dt.float32
    AF = mybir.ActivationFunctionType
    OP = mybir.AluOpType

    nc = bacc.Bacc("TRN2", target_bir_lowering=False, debug=False)
    xT = nc.dram_tensor("xT", [IN_F, B_LOC], f32, kind="ExternalInput")
    w2 = nc.dram_tensor("w2", [NK // PACK * 128, PACK * OUT_F], f16,
                        kind="ExternalInput")
    out = nc.dram_tensor("out", [OUT_F, B_LOC], f32, kind="ExternalOutput")

    GB = GRID * B_LOC  # 4096

    with tile.TileContext(nc) as tc:
        with (
            tc.tile_pool(name="consts", bufs=1) as cpool,
            tc.tile_pool(name="w", bufs=1) as wpool,
            tc.tile_pool(name="x", bufs=2) as xpool,
            tc.tile_pool(name="elem", bufs=3) as epool,
            tc.tile_pool(name="fch", bufs=3) as fpool,
            tc.tile_pool(name="osb", bufs=4) as opool,
            tc.tile_pool(name="ps", bufs=1, space="PSUM") as pspool,
        ):
            k_abs = cfg["k_abs"]
            KB = k_abs * B_LOC          # abs-route span in the g*b free dim

            # Input x tiles first -- nothing can start until these land.
            xTap = xT.ap().rearrange("(t p) b -> t p b", p=128)
            xt32s = []
            for t in range(NT):
                xt32 = xpool.tile([128, B_LOC], f32, tag=f"xt32_{t}",
                                  name=f"xt32_{t}")
                nc.sync.dma_start(out=xt32[:], in_=xTap[t])
                xt32s.append(xt32)

            # Per-partition bias constants -g for optional ACT Abs ops.
            if k_abs > 0:
                gbias = cpool.tile([128, max(k_abs, 1)], f32)
                for g in range(k_abs):
                    nc.vector.memset(gbias[:, g : g + 1], -float(grid_vals[g]))

            # Weights, PACK k-tiles per DMA, in matmul consumption order.
            w2ap = w2.ap().rearrange("(n p) o -> n p o", p=128)
            wslabs = []
            for j in range(NK // PACK):
                ws = wpool.tile([128, PACK * OUT_F], f16, tag=f"w{j}", name=f"w{j}")
                nc.sync.dma_start(out=ws[:], in_=w2ap[j])
                wslabs.append(ws)

            def wslice(kt, ot):
                j, h = KT_SLOT[kt]
                base = h * OUT_F + ot * 128
                return wslabs[j][:, base : base + 128]

            psums = [
                pspool.tile([128, B_LOC], f32, tag=f"ps{ot}", name=f"ps{ot}")
                for ot in range(NO)
            ]

            HG = GRID // 2            # 4 g's per half
            HB = HG * B_LOC           # 2048
            # Route per grid point: edges collapse to single relus of xn;
            # interiors g < k_abs use ACT Abs, the rest DVE min-of-relus.
            route, _sign = _routes(grid_vals, k_abs)
            any_min = any(v == "min" for v in route.values())
            nxns = []
            for t in range(NT):
                xn = xpool.tile([128, B_LOC], f16, tag=f"xn{t}", name=f"xn{t}")
                nc.scalar.activation(xn[:], xt32s[t][:], AF.Tanh)
                if any_min:
                    nxn = xpool.tile([128, B_LOC], f16, tag=f"nxn{t}",
                                     name=f"nxn{t}")
                    nc.vector.tensor_scalar(nxn[:], xn[:], -1.0, None, OP.mult)
                    nxns.append(nxn)
                else:
                    nxns.append(None)

                # xn-channel matmuls first: they only need xn, keeping PE warm
                # while the basis channels are still being computed.
                for ot in range(NO):
                    nc.tensor.matmul(
                        psums[ot][:],
                        wslice(t * NCH + GRID, ot),
                        xn[:],
                        start=(t == 0),
                        stop=False,
                    )

                # Two g-halves per i-tile for finer-grained PE feeding.
                # Edge grid points (g=0 -> -1, g=7 -> +1): the tent collapses,
                #   g0: a = relu(-xn), b = relu(-xn-0.5)  (stored negated)
                #   g7: a = relu(+xn), b = relu(+xn-0.5)  (stored positive)
                # Interior g (1..6): ACT Abs route, A = -(1-d)+, B = -(d-0.5...)
                # A-slot sign is per-channel, absorbed into W2.
                for h in range(2):
                    g0 = h * HG
                    M = epool.tile([128, 2 * HB], f16, tag="M")

                    def asl(k0, k1):  # A-slot span for slots [k0, k1)
                        return slice(k0 * B_LOC, k1 * B_LOC)

                    def bslf(k0, k1):  # B-slot span
                        return slice(HB + k0 * B_LOC, HB + k1 * B_LOC)

                    kinds = [route[g0 + k] for k in range(HG)]
                    # edges: direct single-ts from xn
                    for k, kind in enumerate(kinds):
                        if kind == "e0":
                            # -a = min(xn, 0); -b = min(xn+0.5, 0)
                            nc.vector.tensor_scalar(
                                M[:, asl(k, k + 1)], xn[:], 0.0, 0.0,
                                OP.add, OP.min,
                            )
                            nc.vector.tensor_scalar(
                                M[:, bslf(k, k + 1)], xn[:], 0.5, 0.0,
                                OP.add, OP.min,
                            )
                        elif kind == "e7":
                            # +a = max(xn, 0); +b = max(xn-0.5, 0)
                            nc.vector.tensor_scalar(
                                M[:, asl(k, k + 1)], xn[:], 0.0, 0.0,
                                OP.add, OP.max,
                            )
                            nc.vector.tensor_scalar(
                                M[:, bslf(k, k + 1)], xn[:], 0.5, 0.0,
                                OP.subtract, OP.max,
                            )
                    # abs-route interiors (contiguous slot span)
                    aks = [k for k, kind in enumerate(kinds) if kind == "abs"]
                    if aks:
                        k0, k1 = aks[0], aks[-1] + 1
                        D = epool.tile([128, (k1 - k0) * B_LOC], f16, tag="D")
                        for k in aks:
                            nc.scalar.activation(
                                D[:, (k - k0) * B_LOC : (k - k0 + 1) * B_LOC],
                                xn[:], AF.Abs,
                                bias=gbias[:, g0 + k : g0 + k + 1], scale=1.0,
                            )
                        # mA = min(d-1, 0) = -(1-d)+
                        nc.vector.tensor_scalar(
                            M[:, asl(k0, k1)], D[:], 1.0, 0.0, OP.subtract, OP.min
                        )
                        # y' = min(d-0.5, 0) = -(a-0.5)+
                        nc.vector.tensor_scalar(
                            M[:, bslf(k0, k1)], D[:], 0.5, 0.0, OP.subtract, OP.min
                        )
                    # min-of-relus interiors (contiguous slot span)
                    mks = [k for k, kind in enumerate(kinds) if kind == "min"]
                    if mks:
                        k0, k1 = mks[0], mks[-1] + 1
                        MW = (k1 - k0) * B_LOC
                        R1 = epool.tile([128, MW], f16, tag="R1")
                        R2 = epool.tile([128, MW], f16, tag="R2")
                        for k in mks:
                            gv = float(grid_vals[g0 + k])
                            sl = slice((k - k0) * B_LOC, (k - k0 + 1) * B_LOC)
                            # r1 = relu(1-u) via -xn; r2 = relu(1+u)
                            nc.vector.tensor_scalar(
                                R1[:, sl], nxns[t][:], -(gv + 1.0), 0.0,
                                OP.subtract, OP.max,
                            )
                            nc.vector.tensor_scalar(
                                R2[:, sl], xn[:], gv - 1.0, 0.0,
                                OP.subtract, OP.max,
                            )
                        # +a = min(r1, r2); +b = relu(a - 0.5)
                        nc.vector.tensor_tensor(
                            M[:, asl(k0, k1)], R1[:], R2[:], OP.min
                        )
                        nc.vector.tensor_scalar(
                            M[:, bslf(k0, k1)], M[:, asl(k0, k1)], 0.5, 0.0,
                            OP.subtract, OP.max,
                        )

                    SQ = epool.tile([128, 2 * HB], f16, tag="SQ")
                    nc.scalar.activation(SQ[:, :HB], M[:, :HB], AF.Square,
                                         scale=SQRT2)
                    nc.scalar.activation(SQ[:, HB:], M[:, HB:], AF.Square,
                                         scale=SQRT8)

                    # CU_A = 2a^3 (sign follows A-slot), CU_B = 8b^3
                    CU = epool.tile([128, 2 * HB], f16, tag="CU")
                    nc.vector.tensor_tensor(CU[:], SQ[:], M[:], OP.mult)

                    # F = CU_A - CU_B = +-(2a^3 - 8b^3) = +-f
                    F = fpool.tile([128, HB], f16, tag="F")
                    nc.vector.tensor_tensor(F[:], CU[:, :HB], CU[:, HB:],
                                            OP.subtract)

                    last_half = t == NT - 1 and h == 1
                    for ot in range(NO):
                        for gg in range(HG):
                            ch = g0 + gg
                            rhs = F[:, gg * B_LOC : (gg + 1) * B_LOC]
                            kt = t * NCH + ch
                            nc.tensor.matmul(
                                psums[ot][:],
                                wslice(kt, ot),
                                rhs,
                                start=False,
                                stop=(last_half and ch == GRID - 1),
                            )
                        if last_half:
                            # This psum chain just closed: drain it to DRAM
                            # immediately so copies/stores overlap the
                            # remaining chains' matmuls.
                            osb = opool.tile([128, B_LOC], f32, tag="osb",
                                             name=f"osb{ot}")
                            if cfg["copy_on_act"]:
                                nc.scalar.copy(osb[:], psums[ot][:])
                            else:
                                nc.vector.tensor_copy(osb[:], psums[ot][:])
                            nc.sync.dma_start(
                                out=out.ap()[ot * 128 : (ot + 1) * 128, :],
                                in_=osb[:],
                            )

    nc.compile()
    return nc


def _get_nc(grid_vals, cfg=None):
    cfg = cfg or CFG
    key = (tuple(np.asarray(grid_vals, np.float32).tolist()), tuple(sorted(cfg.items())))
    if key not in _CACHE:
        _CACHE[key] = _build(grid_vals, cfg)
    return _CACHE[key]


def _prep_inputs(x, spline_weight, base_activation, grid_vals, k_abs):
    x = np.asarray(x, np.float32)
    sw = np.asarray(spline_weight, np.float32)
    ba = np.asarray(base_activation, np.float32)
    # W2[k, o] with k = (t*NCH + ch)*128 + p ; ch<8 -> +-sw[o, i, g] ; ch==8 -> 0.1*ba[o, i]
    # abs-route channels (g < k_abs) produce -f, so their weights are negated.
    W2 = np.empty((NK, 128, OUT_F), np.float32)
    sw_t = sw.transpose(1, 2, 0)  # [in, g, out]
    ba_t = ba.T  # [in, out]
    _route, sign = _routes(grid_vals, k_abs)
    for t in range(NT):
        isl = slice(t * 128, (t + 1) * 128)
        for g in range(GRID):
            W2[t * NCH + g] = sign[g] * sw_t[isl, g, :]
        W2[t * NCH + GRID] = 0.1 * ba_t[isl, :]
    # Pack PACK k-tiles per DMA slab, in matmul consumption order:
    # slab j, partition p holds [W2[ord[j*PACK+h]][p] for h in 0..PACK-1].
    W2p = np.empty((NK // PACK, 128, PACK * OUT_F), np.float32)
    for j in range(NK // PACK):
        for h in range(PACK):
            W2p[j, :, h * OUT_F : (h + 1) * OUT_F] = W2[CONSUME_ORDER[j * PACK + h]]
    W2 = W2p.reshape(NK // PACK * 128, PACK * OUT_F).astype(np.float16)
    xT = np.ascontiguousarray(x.T)  # [IN_F, BATCH]
    in_maps = [
        {
            "xT": np.ascontiguousarray(xT[:, c * B_LOC : (c + 1) * B_LOC]),
            "w2": W2,
        }
        for c in range(NCORES)
    ]
    return in_maps


def _run(x, spline_weight, base_activation, grid_points, trace=False, cfg=None,
         tmpdir=None):
    from concourse.bass_utils import run_bass_kernel_spmd

    nc = _get_nc(np.asarray(grid_points, np.float32), cfg)
    in_maps = _prep_inputs(x, spline_weight, base_activation,
                           np.asarray(grid_points, np.float32),
                           (cfg or CFG)["k_abs"])
    res = run_bass_kernel_spmd(
        nc, in_maps, list(range(NCORES)), trace=trace, tmpdir=tmpdir
    )
    outs = [res.results[c]["out"] for c in range(NCORES)]  # each [OUT_F, B_LOC]
    full = np.concatenate(outs, axis=1)  # [OUT_F, BATCH]
    return np.ascontiguousarray(full.T.astype(np.float32)), res


def kernel(x, spline_weight, base_activation, grid_points):
    out, _ = _run(x, spline_weight, base_activation, grid_points)
    return out


# revision 36
# speedup vs baseline: 1.1553x; 1.1553x over previous
"""Trainium2 Bass kernel for AdvancedKANLayer.

Math (per reference):
  xn = tanh(x)                                  (B, IN)
  d_g = |xn - g|                                for 8 grid points g
  f(d) = 2*(1-d)+^3 - 8*(0.5-d)+^3              (piecewise-cubic B-spline basis)
  out[b,o] = sum_{i,g} f(d_g[b,i]) * sw[o,i,g] + 0.1 * xn @ ba.T

Device formulation (per core, batch-sharded 8 ways, b_loc=512), with
a = (1-|u|)+ (tent, u = xn-g) and b = (a-0.5)+ so that f = 2a^3 - 8b^3:
  - edge grid points (g=-1, +1): tent collapses, a = relu(-+xn),
    b = relu(-+xn-0.5) -- one DVE tensor_scalar (4x mode) each.
  - interior g < k_abs (ACT Abs route): d = Abs(xn-g) on ScalarE, then
    -a = min(d-1, 0), -b = min(d-0.5, 0) via DVE tensor_scalar (4x).
  - interior g >= k_abs (min-of-relus route): r1 = relu(-xn+g+1),
    r2 = relu(xn-g+1) (DVE ts 4x), a = min(r1, r2) (tt 2x),
    b = relu(a-0.5) (ts 4x).
  sA = Square(sqrt(2)*(+-a)) = 2a^2, sB = Square(sqrt(8)*(+-b)) = 8b^2 on
  ScalarE (all constant folds ride the free Square input scale), then
  CU = SQ*M (tt 2x) gives +-2a^3 / +-8b^3 and F = CU_A - CU_B = +-f (tt 2x).
  Per-channel signs are absorbed into the weights.
  out = W2.T @ [F channels (8 per i-tile), xn channel] -- single fp16 PE
  contraction, K = 4*(8+1)*128 = 4608, 144 matmuls of N=512 accumulating
  in 4 PSUM banks. W2 = [+-sw | 0.1*ba].

The work per i-tile is split into two 4-grid halves so PE bursts interleave
with the elementwise pipeline (keeps the PE HAM warm-ish). Weight k-tiles
are packed 4-per-DMA in consumption order so the first matmuls unblock
early. Layout: i on partitions (4 tiles of 128), b on free dim (512). x is
passed transposed per core: xT[i, b]. Output is [o, b] per core, gathered +
transposed on host. Elementwise tensors fp16; matmul fp16 -> fp32 PSUM.
Measured: ~70us/core HW exec, rel err ~7e-4 vs the fp32 reference.
"""

import sys

if "/opt/trn_rl_repo" not in sys.path:
    sys.path.insert(0, "/opt/trn_rl_repo")

import numpy as np

IN_F = 512
OUT_F = 512
GRID = 8
BATCH = 4096
NCORES = 8
B_LOC = BATCH // NCORES  # 512
NT = IN_F // 128         # 4 i-tiles
NO = OUT_F // 128        # 4 o-tiles
NCH = GRID + 1           # 8 basis channels + 1 xn channel per i-tile
NK = NT * NCH            # 36 k-tiles of 128

CFG = {
    "k_abs": 5,          # g's 0..k_abs-1 use ACT Abs route; rest use DVE min-of-relus
    "sq_on_act": 2,      # 0..2 of the two Square layers on ScalarE (rest DVE stt)
    "copy_on_act": False, # PSUM->SBUF output copies on ScalarE
}

# Weight k-tiles are DMA'd packed PACK-at-a-time (PACK*1KB contiguous DRAM rows)
# in the order the matmuls consume them: per i-tile, the xn channel first.
PACK = 4
CONSUME_ORDER = []
for _t in range(NT):
    CONSUME_ORDER.append(_t * NCH + GRID)
    CONSUME_ORDER.extend(_t * NCH + _g for _g in range(GRID))
KT_SLOT = {kt: (j // PACK, j % PACK) for j, kt in enumerate(CONSUME_ORDER)}

_CACHE = {}

SQRT2 = float(np.sqrt(2.0))
SQRT8 = float(np.sqrt(8.0))


def _routes(grid_vals, k_abs):
    """Per-grid-point compute route and channel sign (shared by the kernel
    builder and the weight prep so they can never disagree).

    The edge shortcut relies on grid[0] = -1 and grid[-1] = +1 (with
    xn = tanh(x) strictly inside (-1, 1)); fall back to the Abs route if a
    different grid ever shows up.
    """
    route = {}
    sign = {}
    for g in range(GRID):
        if g == 0 and abs(float(grid_vals[0]) + 1.0) < 1e-6:
            route[g], sign[g] = "e0", -1.0
        elif g == GRID - 1 and abs(float(grid_vals[GRID - 1]) - 1.0) < 1e-6:
            route[g], sign[g] = "e7", 1.0
        elif 0 < g < min(k_abs, GRID - 1):
            route[g], sign[g] = "abs", -1.0
        else:
            route[g], sign[g] = "min", 1.0
    return route, sign


def _build(grid_vals, cfg):
    import concourse.tile as tile
    import concourse.mybir as mybir
    from concourse import bacc

    dt = mybir.dt
    f16 = dt.float16
    f32 = dt.float32
    AF = mybir.ActivationFunctionType
    OP = mybir.AluOpType

    nc = bacc.Bacc("TRN2", target_bir_lowering=False, debug=False)
    xT = nc.dram_tensor("xT", [IN_F, B_LOC], f32, kind="ExternalInput")
    w2 = nc.dram_tensor("w2", [NK // PACK * 128, PACK * OUT_F], f16,
                        kind="ExternalInput")
    out = nc.dram_tensor("out", [OUT_F, B_LOC], f32, kind="ExternalOutput")

    GB = GRID * B_LOC  # 4096

    with tile.TileContext(nc) as tc:
        with (
            tc.tile_pool(name="consts", bufs=1) as cpool,
            tc.tile_pool(name="w", bufs=1) as wpool,
            tc.tile_pool(name="x", bufs=2) as xpool,
            tc.tile_pool(name="elem", bufs=3) as epool,
            tc.tile_pool(name="fch", bufs=3) as fpool,
            tc.tile_pool(name="osb", bufs=2) as opool,
            tc.tile_pool(name="ps", bufs=1, space="PSUM") as pspool,
        ):
            k_abs = cfg["k_abs"]
            KB = k_abs * B_LOC          # abs-route span in the g*b free dim

            # Input x tiles first -- nothing can start until these land.
            xTap = xT.ap().rearrange("(t p) b -> t p b", p=128)
            xt32s = []
            for t in range(NT):
                xt32 = xpool.tile([128, B_LOC], f32, tag=f"xt32_{t}",
                                  name=f"xt32_{t}")
                nc.sync.dma_start(out=xt32[:], in_=xTap[t])
                xt32s.append(xt32)

            # Per-partition bias constants -g for optional ACT Abs ops.
            if k_abs > 0:
                gbias = cpool.tile([128, max(k_abs, 1)], f32)
                for g in range(k_abs):
                    nc.vector.memset(gbias[:, g : g + 1], -float(grid_vals[g]))

            # Weights, PACK k-tiles per DMA, in matmul consumption order.
            w2ap = w2.ap().rearrange("(n p) o -> n p o", p=128)
            wslabs = []
            for j in range(NK // PACK):
                ws = wpool.tile([128, PACK * OUT_F], f16, tag=f"w{j}", name=f"w{j}")
                nc.sync.dma_start(out=ws[:], in_=w2ap[j])
                wslabs.append(ws)

            def wslice(kt, ot):
                j, h = KT_SLOT[kt]
                base = h * OUT_F + ot * 128
                return wslabs[j][:, base : base + 128]

            psums = [
                pspool.tile([128, B_LOC], f32, tag=f"ps{ot}", name=f"ps{ot}")
                for ot in range(NO)
            ]

            HG = GRID // 2            # 4 g's per half
            HB = HG * B_LOC           # 2048
            # Route per grid point: edges collapse to single relus of xn;
            # interiors g < k_abs use ACT Abs, the rest DVE min-of-relus.
            route, _sign = _routes(grid_vals, k_abs)
            any_min = any(v == "min" for v in route.values())
            nxns = []
            for t in range(NT):
                xn = xpool.tile([128, B_LOC], f16, tag=f"xn{t}", name=f"xn{t}")
                nc.scalar.activation(xn[:], xt32s[t][:], AF.Tanh)
                if any_min:
                    nxn = xpool.tile([128, B_LOC], f16, tag=f"nxn{t}",
                                     name=f"nxn{t}")
                    nc.vector.tensor_scalar(nxn[:], xn[:], -1.0, None, OP.mult)
                    nxns.append(nxn)
                else:
                    nxns.append(None)

                # xn-channel matmuls first: they only need xn, keeping PE warm
                # while the basis channels are still being computed.
                for ot in range(NO):
                    nc.tensor.matmul(
                        psums[ot][:],
                        wslice(t * NCH + GRID, ot),
                        xn[:],
                        start=(t == 0),
                        stop=False,
                    )

                # Two g-halves per i-tile for finer-grained PE feeding.
                # Edge grid points (g=0 -> -1, g=7 -> +1): the tent collapses,
                #   g0: a = relu(-xn), b = relu(-xn-0.5)  (stored negated)
                #   g7: a = relu(+xn), b = relu(+xn-0.5)  (stored positive)
                # Interior g (1..6): ACT Abs route, A = -(1-d)+, B = -(d-0.5...)
                # A-slot sign is per-channel, absorbed into W2.
                for h in range(2):
                    g0 = h * HG
                    M = epool.tile([128, 2 * HB], f16, tag="M")

                    def asl(k0, k1):  # A-slot span for slots [k0, k1)
                        return slice(k0 * B_LOC, k1 * B_LOC)

                    def bslf(k0, k1):  # B-slot span
                        return slice(HB + k0 * B_LOC, HB + k1 * B_LOC)

                    kinds = [route[g0 + k] for k in range(HG)]
                    # edges: direct single-ts from xn
                    for k, kind in enumerate(kinds):
                        if kind == "e0":
                            # -a = min(xn, 0); -b = min(xn+0.5, 0)
                            nc.vector.tensor_scalar(
                                M[:, asl(k, k + 1)], xn[:], 0.0, 0.0,
                                OP.add, OP.min,
                            )
                            nc.vector.tensor_scalar(
                                M[:, bslf(k, k + 1)], xn[:], 0.5, 0.0,
                                OP.add, OP.min,
                            )
                        elif kind == "e7":
                            # +a = max(xn, 0); +b = max(xn-0.5, 0)
                            nc.vector.tensor_scalar(
                                M[:, asl(k, k + 1)], xn[:], 0.0, 0.0,
                                OP.add, OP.max,
                            )
                            nc.vector.tensor_scalar(
                                M[:, bslf(k, k + 1)], xn[:], 0.5, 0.0,
                                OP.subtract, OP.max,
                            )
                    # abs-route interiors (contiguous slot span)
                    aks = [k for k, kind in enumerate(kinds) if kind == "abs"]
                    if aks:
                        k0, k1 = aks[0], aks[-1] + 1
                        D = epool.tile([128, (k1 - k0) * B_LOC], f16, tag="D")
                        for k in aks:
                            nc.scalar.activation(
                                D[:, (k - k0) * B_LOC : (k - k0 + 1) * B_LOC],
                                xn[:], AF.Abs,
                                bias=gbias[:, g0 + k : g0 + k + 1], scale=1.0,
                            )
                        # mA = min(d-1, 0) = -(1-d)+
                        nc.vector.tensor_scalar(
                            M[:, asl(k0, k1)], D[:], 1.0, 0.0, OP.subtract, OP.min
                        )
                        # y' = min(d-0.5, 0) = -(a-0.5)+
                        nc.vector.tensor_scalar(
                            M[:, bslf(k0, k1)], D[:], 0.5, 0.0, OP.subtract, OP.min
                        )
                    # min-of-relus interiors (contiguous slot span)
                    mks = [k for k, kind in enumerate(kinds) if kind == "min"]
                    if mks:
                        k0, k1 = mks[0], mks[-1] + 1
                        MW = (k1 - k0) * B_LOC
                        R1 = epool.tile([128, MW], f16, tag="R1")
                        R2 = epool.tile([128, MW], f16, tag="R2")
                        for k in mks:
                            gv = float(grid_vals[g0 + k])
                            sl = slice((k - k0) * B_LOC, (k - k0 + 1) * B_LOC)
                            # r1 = relu(1-u) via -xn; r2 = relu(1+u)
                            nc.vector.tensor_scalar(
                                R1[:, sl], nxns[t][:], -(gv + 1.0), 0.0,
                                OP.subtract, OP.max,
                            )
                            nc.vector.tensor_scalar(
                                R2[:, sl], xn[:], gv - 1.0, 0.0,
                                OP.subtract, OP.max,
                            )
                        # +a = min(r1, r2); +b = relu(a - 0.5)
                        nc.vector.tensor_tensor(
                            M[:, asl(k0, k1)], R1[:], R2[:], OP.min
                        )
                        nc.vector.tensor_scalar(
                            M[:, bslf(k0, k1)], M[:, asl(k0, k1)], 0.5, 0.0,
                            OP.subtract, OP.max,
                        )

                    SQ = epool.tile([128, 2 * HB], f16, tag="SQ")
                    nc.scalar.activation(SQ[:, :HB], M[:, :HB], AF.Square,
                                         scale=SQRT2)
                    nc.scalar.activation(SQ[:, HB:], M[:, HB:], AF.Square,
                                         scale=SQRT8)

                    # CU_A = 2a^3 (sign follows A-slot), CU_B = 8b^3
                    CU = epool.tile([128, 2 * HB], f16, tag="CU")
                    nc.vector.tensor_tensor(CU[:], SQ[:], M[:], OP.mult)

                    # F = CU_A - CU_B = +-(2a^3 - 8b^3) = +-f
                    F = fpool.tile([128, HB], f16, tag="F")
                    nc.vector.tensor_tensor(F[:], CU[:, :HB], CU[:, HB:],
                                            OP.subtract)

                    for ot in range(NO):
                        for gg in range(HG):
                            ch = g0 + gg
                            rhs = F[:, gg * B_LOC : (gg + 1) * B_LOC]
                            kt = t * NCH + ch
                            nc.tensor.matmul(
                                psums[ot][:],
                                wslice(kt, ot),
                                rhs,
                                start=False,
                                stop=(t == NT - 1 and ch == GRID - 1),
                            )

            for ot in range(NO):
                if cfg["copy_on_act"] == "dma":
                    # DMA straight from PSUM to DRAM
                    nc.sync.dma_start(
                        out=out.ap()[ot * 128 : (ot + 1) * 128, :],
                        in_=psums[ot][:],
                    )
                    continue
                osb = opool.tile([128, B_LOC], f32, tag="osb")
                if cfg["copy_on_act"]:
                    nc.scalar.copy(osb[:], psums[ot][:])
                else:
                    nc.vector.tensor_copy(osb[:], psums[ot][:])
                nc.sync.dma_start(
                    out=out.ap()[ot * 128 : (ot + 1) * 128, :], in_=osb[:]
                )

    nc.compile()
    return nc


def _get_nc(grid_vals, cfg=None):
    cfg = cfg or CFG
    key = (tuple(np.asarray(grid_vals, np.float32).tolist()), tuple(sorted(cfg.items())))
    if key not in _CACHE:
        _CACHE[key] = _build(grid_vals, cfg)
    return _CACHE[key]


def _prep_inputs(x, spline_weight, base_activation, grid_vals, k_abs):
    x = np.asarray(x, np.float32)
    sw = np.asarray(spline_weight, np.float32)
    ba = np.asarray(base_activation, np.float32)
    # W2[k, o] with k = (t*NCH + ch)*128 + p ; ch<8 -> +-sw[o, i, g] ; ch==8 -> 0.1*ba[o, i]
    # abs-route channels (g < k_abs) produce -f, so their weights are negated.
    W2 = np.empty((NK, 128, OUT_F), np.float32)
    sw_t = sw.transpose(1, 2, 0)  # [in, g, out]
    ba_t = ba.T  # [in, out]
    _route, sign = _routes(grid_vals, k_abs)
    for t in range(NT):
        isl = slice(t * 128, (t + 1) * 128)
        for g in range(GRID):
            W2[t * NCH + g] = sign[g] * sw_t[isl, g, :]
        W2[t * NCH + GRID] = 0.1 * ba_t[isl, :]
    # Pack PACK k-tiles per DMA slab, in matmul consumption order:
    # slab j, partition p holds [W2[ord[j*PACK+h]][p] for h in 0..PACK-1].
    W2p = np.empty((NK // PACK, 128, PACK * OUT_F), np.float32)
    for j in range(NK // PACK):
        for h in range(PACK):
            W2p[j, :, h * OUT_F : (h + 1) * OUT_F] = W2[CONSUME_ORDER[j * PACK + h]]
    W2 = W2p.reshape(NK // PACK * 128, PACK * OUT_F).astype(np.float16)
    xT = np.ascontiguousarray(x.T)  # [IN_F, BATCH]
    in_maps = [
        {
            "xT": np.ascontiguousarray(xT[:, c * B_LOC : (c + 1) * B_LOC]),
            "w2": W2,
        }
        for c in range(NCORES)
    ]
    return in_maps


def _run(x, spline_weight, base_activation, grid_points, trace=False, cfg=None,
         tmpdir=None):
    from concourse.bass_utils import run_bass_kernel_spmd

    nc = _get_nc(np.asarray(grid_points, np.float32), cfg)
    in_maps = _prep_inputs(x, spline_weight, base_activation,
                           np.asarray(grid_points, np.float32),
                           (cfg or CFG)["k_abs"])
    res = run_bass_kernel_spmd(
        nc, in_maps, list(range(NCORES)), trace=trace, tmpdir=tmpdir
    )
    outs = [res.results[c]["out"] for c in range(NCORES)]  # each [OUT_F, B_LOC]
    full = np.concatenate(outs, axis=1)  # [OUT_F, BATCH]
    return np.ascontiguousarray(full.T.astype(np.float32)), res


def kernel(x, spline_weight, base_activation, grid_points):
    out, _ = _run(x, spline_weight, base_activation, grid_points)
    return out
